# revision 8
# baseline (speedup 1.0000x reference)
"""Trainium2 Bass kernel for nn_AnalyticalStage2 (v4).

Math (per batch row b, time index i, constant per-row decay d):
    v_i = d*v_{i-1} + p_i,   omega_i = A*p_i + c*v_{i-1},  c = D*(1-d)

Pair reformulation (halves the serial DVE scan):
    w_k := v_{2k+1} satisfies  w_k = d^2 * w_{k-1} + u_k,
    u_k  = d*pe_k + po_k          (pe=p_even, po=p_odd)
    om_e_k = A*pe_k + c*w_{k-1}
    om_o_k = A*po_k + c*d*w_{k-1} + c*pe_k

Mapping: 512 rows -> 8 cores x 64 rows. Per core, partitions = 2 time
halves x 64 rows (q = h*64 + b); per-partition sequence = 8192 pairs.
Host stages p as bf16 deinterleaved [q, parity*8192 + k]; the output is
staged bf16 TILE-INTERLEAVED: x = 2048*t + 1024*e + (k - 1024*t), i.e.
per 1024-pair tile the even block then the odd block. Host re-
interleaves + upcasts.

Per W=1024 tile: PE computes u into PSUM (diag(d)@pe + I@po) and the
combine into one merged [128,2048] PSUM tile (stationary-batched runs:
A x4, c x4, cd x2 + u's d x2, I x2); DVE runs tensor_tensor_scan into a
persistent wbuf (w_shift = wbuf[:, lo:lo+W], no boundary copies); ACT
drains the merged PSUM in ONE op per tile. Inputs ride HWDGE/Q1
(nc.sync), outputs SWDGE/Q0 (nc.gpsimd). PE pre-warmed ~12 junk MMs.

Half 2 scans from 0; tail fixup om2 += (c*v1e) * G2ti where G2ti is the
tile-interleaved geometric table with d PRE-FOLDED into odd blocks
(G2ti[2048t+1024e+j] = d^e * (d^2)^(1024t+j)), so one per-partition
scalar serves both parities: DVE tensor_scalar (4x) + tensor_add (2x)
in widening chunks overlapped with out-DMAs.
"""

import numpy as np
import ml_dtypes

import concourse.bass as bass
import concourse.bacc as bacc
import concourse.mybir as mybir
from concourse.bass_utils import run_bass_kernel_spmd
from concourse.tile import TileContext

_C = 0.206756
B, NT = 512, 32768
NCORES = 8
BLOC = B // NCORES  # 64
DELTA = 0.2 / (NT - 1)

F32 = mybir.dt.float32
BF16 = mybir.dt.bfloat16
ALU = mybir.AluOpType
ACTF = mybir.ActivationFunctionType

TH = NT // 2  # half length 16384
NK = TH // 2  # pairs per half 8192
W = 1024  # compute tile width (pairs)
NTILES = NK // W  # 8
MM = 512  # matmul free-dim chunk (one PSUM bank)
NWARM = 12  # PE warmup matmuls (~3.5us at cold clock incl. LDW)

# input DMA chunks per parity: (lo, width) in pairs
IN_CHUNKS = [(0, 1024), (1024, 3072), (4096, 4096)]

BF = ml_dtypes.bfloat16


def build(nc):
    p_ext = nc.declare_dram_parameter("p", [128, 2 * NK], BF16, isOutput=False)
    hr_ext = nc.declare_dram_parameter("h_raw", [128, 128], F32, isOutput=False)
    out_ext = nc.declare_dram_parameter("out", [128, 2 * NK], BF16, isOutput=True)

    with TileContext(nc) as tc:
        with (
            tc.tile_pool(name="const", bufs=1) as cpool,
            tc.tile_pool(name="big", bufs=1) as bigpool,
            tc.tile_pool(name="pb", bufs=2) as bpool,
            tc.tile_pool(name="st", bufs=3) as stpool,
            tc.tile_pool(name="psu", bufs=2, space="PSUM") as psu,
            tc.tile_pool(name="pso", bufs=1, space="PSUM") as pso,
        ):
            # ---- PE warmup: junk matmuls to flip HAM to 8/8 early ----
            wz = cpool.tile([128, 128], BF16)
            nc.vector.memset(wz[:, :], 0.0)
            warm = psu.tile([128, W], F32, tag="u")
            for _ in range(NWARM):
                nc.tensor.matmul(
                    warm[:, 0:128], wz[:], wz[:, 0:128], start=True, stop=True
                )

            # ---- input DMAs: params on Q0 (SWDGE), p stream on Q1 (HWDGE) ----
            hr = cpool.tile([128, 128], F32)
            nc.gpsimd.dma_start(out=hr[:, :], in_=hr_ext[:])

            pch = []  # [(e, lo, width, tile), ...]
            for lo, wd in IN_CHUNKS:
                for e in range(2):
                    t = bpool.tile([128, wd], BF16, tag=f"pb{wd}")
                    nc.sync.dma_start(
                        out=t[:, :], in_=p_ext[:, e * NK + lo : e * NK + lo + wd]
                    )
                    pch.append((e, lo, wd, t))

            def pslice(e, lo, width):
                for pe_, clo, cw, tl in pch:
                    if pe_ == e and clo <= lo and lo + width <= clo + cw:
                        return tl[:, lo - clo : lo - clo + width]
                raise AssertionError((e, lo, width))

            # ---- params on all 128 partitions ----
            E1, E2, eta = hr[:, 0:1], hr[:, 1:2], hr[:, 2:3]
            prm = cpool.tile([128, 16], F32)

            def pc(i):
                return prm[:, i : i + 1]

            s, se, rse, e12 = pc(0), pc(1), pc(2), pc(3)
            alpha, lnd, d, rs = pc(4), pc(5), pc(6), pc(7)
            A, rE2, t2, t3 = pc(8), pc(9), pc(10), pc(11)
            D, omd, c, dd = pc(12), pc(13), pc(14), pc(15)

            nc.vector.tensor_add(out=s, in0=E1, in1=E2)
            nc.vector.tensor_mul(out=se, in0=s, in1=eta)
            nc.vector.reciprocal(rse, se)
            nc.vector.tensor_mul(out=e12, in0=E1, in1=E2)
            nc.vector.tensor_mul(out=alpha, in0=e12, in1=rse)
            nc.vector.tensor_scalar_mul(lnd, alpha, -DELTA)
            nc.scalar.activation(d, lnd, ACTF.Exp)
            nc.vector.reciprocal(rs, s)
            nc.vector.tensor_scalar_mul(A, rs, _C)
            nc.vector.reciprocal(rE2, E2)
            nc.vector.tensor_mul(out=t2, in0=E1, in1=rE2)
            nc.vector.tensor_mul(out=t3, in0=t2, in1=rs)
            nc.vector.tensor_scalar_mul(D, t3, _C)
            nc.vector.tensor_scalar(omd, d, -1.0, 1.0, ALU.mult, ALU.add)
            nc.vector.tensor_mul(out=c, in0=D, in1=omd)
            nc.vector.tensor_mul(out=dd, in0=d, in1=d)

            prm2 = cpool.tile([128, 4], F32)
            cd = prm2[:, 0:1]
            lndd = prm2[:, 1:2]
            nc.vector.tensor_mul(out=cd, in0=c, in1=d)
            nc.vector.tensor_scalar_mul(lndd, lnd, 2.0)

            # dks2[j] = dd^(2^j), j=0..12 (for G2 doublings)
            dks2 = cpool.tile([128, 13], F32)
            nc.scalar.copy(out=dks2[:, 0:1], in_=dd)
            for j in range(1, 13):
                nc.vector.tensor_mul(
                    out=dks2[:, j : j + 1],
                    in0=dks2[:, j - 1 : j],
                    in1=dks2[:, j - 1 : j],
                )

            # 0/1 identity mask -> bf16 diag stationaries
            I01 = cpool.tile([128, 128], F32)
            one = cpool.tile([128, 1], F32)
            nc.vector.memset(one[:, :], 1.0)
            nc.gpsimd.affine_select(
                out=I01[:],
                in_=one[:, 0:1].broadcast_to([128, 128]),
                pattern=[[1, 128]],
                compare_op=ALU.is_equal,
                fill=0.0,
                base=0,
                channel_multiplier=-1,
            )
            diag_d = cpool.tile([128, 128], BF16)
            diag_A = cpool.tile([128, 128], BF16)
            diag_c = cpool.tile([128, 128], BF16)
            diag_cd = cpool.tile([128, 128], BF16)
            ident = cpool.tile([128, 128], BF16)
            nc.vector.tensor_scalar_mul(diag_d[:], I01[:], d)
            nc.vector.tensor_scalar_mul(diag_A[:], I01[:], A)
            nc.vector.tensor_scalar_mul(diag_c[:], I01[:], c)
            nc.vector.tensor_scalar_mul(diag_cd[:], I01[:], cd)
            nc.scalar.copy(out=ident[:, :], in_=I01[:])

            # ---- G2ti: tile-interleaved geometric table (d folded into odd
            # blocks): G2ti[:, 2048t + 1024e + j] = d^e * dd^(1024t + j) ----
            ramp = cpool.tile([128, 1024], F32)
            nc.gpsimd.iota(
                out=ramp[:],
                pattern=[[1, 1024]],
                base=0,
                channel_multiplier=0,
                allow_small_or_imprecise_dtypes=True,
            )
            G2 = bigpool.tile([128, 2 * NK], BF16)
            nc.scalar.activation(G2[:, 0:1024], ramp[:], ACTF.Exp, scale=lndd)
            nc.scalar.activation(G2[:, 1024:2048], G2[:, 0:1024], ACTF.Copy, scale=d)
            kk = 2048
            for j in (10, 11, 12):
                nc.scalar.activation(
                    G2[:, kk : 2 * kk],
                    G2[:, 0:kk],
                    ACTF.Copy,
                    scale=dks2[:, j : j + 1],
                )
                kk *= 2

            # persistent buffers
            ombuf = bigpool.tile([128, 2 * NK], BF16)  # tile-interleaved om
            wbuf = bigpool.tile([128, NK + 1], BF16)  # w with leading zero
            nc.vector.memset(wbuf[:, 0:1], 0.0)

            def u_mms(t, ups):
                lo = t * W
                pe = pslice(0, lo, W)
                po = pslice(1, lo, W)
                for q in range(W // MM):
                    nc.tensor.matmul(
                        ups[:, q * MM : (q + 1) * MM],
                        diag_d[:],
                        pe[:, q * MM : (q + 1) * MM],
                        start=True,
                        stop=False,
                    )
                for q in range(W // MM):
                    nc.tensor.matmul(
                        ups[:, q * MM : (q + 1) * MM],
                        ident[:],
                        po[:, q * MM : (q + 1) * MM],
                        start=False,
                        stop=True,
                    )

            u_tiles = {}
            u0 = psu.tile([128, W], F32, tag="u")
            u_mms(0, u0)
            u_tiles[0] = u0

            # ---- main loop ----
            for t in range(NTILES):
                lo = t * W
                ups = u_tiles.pop(t)

                if t + 1 < NTILES:
                    un = psu.tile([128, W], F32, tag="u")
                    u_mms(t + 1, un)
                    u_tiles[t + 1] = un

                nc.vector.tensor_tensor_scan(
                    out=wbuf[:, lo + 1 : lo + W + 1],
                    data0=dd.broadcast_to([128, W]),
                    data1=ups[:],
                    initial=wbuf[:, lo : lo + 1],
                    op0=ALU.mult,
                    op1=ALU.add,
                )

                pe = pslice(0, lo, W)
                po = pslice(1, lo, W)
                wsh = wbuf[:, lo : lo + W]
                omps = pso.tile([128, 2 * W], F32, tag="om")
                # stationary-batched runs: A x4, c x4, cd x2
                for q in range(W // MM):
                    nc.tensor.matmul(
                        omps[:, q * MM : (q + 1) * MM],
                        diag_A[:],
                        pe[:, q * MM : (q + 1) * MM],
                        start=True,
                        stop=False,
                    )
                for q in range(W // MM):
                    nc.tensor.matmul(
                        omps[:, W + q * MM : W + (q + 1) * MM],
                        diag_A[:],
                        po[:, q * MM : (q + 1) * MM],
                        start=True,
                        stop=False,
                    )
                for q in range(W // MM):
                    nc.tensor.matmul(
                        omps[:, q * MM : (q + 1) * MM],
                        diag_c[:],
                        wsh[:, q * MM : q * MM + MM],
                        start=False,
                        stop=True,
                    )
                for q in range(W // MM):
                    nc.tensor.matmul(
                        omps[:, W + q * MM : W + (q + 1) * MM],
                        diag_c[:],
                        pe[:, q * MM : (q + 1) * MM],
                        start=False,
                        stop=False,
                    )
                for q in range(W // MM):
                    nc.tensor.matmul(
                        omps[:, W + q * MM : W + (q + 1) * MM],
                        diag_cd[:],
                        wsh[:, q * MM : q * MM + MM],
                        start=False,
                        stop=True,
                    )

                # one drain per tile into the tile-interleaved ombuf
                nc.scalar.copy(out=ombuf[:, 2 * lo : 2 * lo + 2 * W], in_=omps[:])

                # stream half-1 rows out every 4 tiles (SWDGE / Q0)
                if t % 4 == 3:
                    g = 2 * (lo + W) - 8192
                    nc.gpsimd.dma_start(
                        out=out_ext[0:64, g : g + 8192],
                        in_=ombuf[0:64, g : g + 8192],
                    )

            # ---- tail: fix up half 2 (rows 64:128) ----
            v1e = cpool.tile([128, 1], F32)
            nc.gpsimd.dma_start(out=v1e[64:128, :], in_=wbuf[0:64, NK : NK + 1])
            cv64 = prm2[64:128, 2:3]
            nc.vector.tensor_mul(out=cv64, in0=prm[64:128, 14:15], in1=v1e[64:128, :])

            # widening chunks over the tile-interleaved x axis
            CHUNKS = [(0, 4096), (4096, 4096), (8192, 4096), (12288, 4096)]
            for lo, cw in CHUNKS:
                fix = stpool.tile([128, 4096], BF16, tag="fix")
                stage = stpool.tile([128, 4096], BF16, tag="stage")
                nc.vector.tensor_scalar_mul(
                    fix[64:128, 0:cw], G2[64:128, lo : lo + cw], cv64
                )
                nc.vector.tensor_add(
                    out=stage[64:128, 0:cw],
                    in0=fix[64:128, 0:cw],
                    in1=ombuf[64:128, lo : lo + cw],
                )
                nc.gpsimd.dma_start(
                    out=out_ext[64:128, lo : lo + cw],
                    in_=stage[64:128, 0:cw],
                )

    return nc


def make_nc():
    nc = bacc.Bacc(None)
    build(nc)
    nc.finalize()
    return nc


def _stage_p(p_core):
    # [64, 32768] f32 -> [128, 16384] bf16: q=h*64+b, x=e*8192+k
    x = np.asarray(p_core, dtype=BF).reshape(64, 2, NK, 2)
    return np.ascontiguousarray(x.transpose(1, 0, 3, 2).reshape(128, 2 * NK))


def _stage_hr(hr_core):
    # [64, 3] f32 -> [128, 128] f32 (rows duplicated across halves, padded)
    out = np.zeros((128, 128), dtype=np.float32)
    out[0:64, 0:3] = hr_core
    out[64:128, 0:3] = hr_core
    return out


def _unstage_out(o_core):
    # [128, 16384] bf16 tile-interleaved -> [64, 32768] f32
    # x = 2048*t + 1024*e + j  <->  (h, b) time i = h*16384 + 2*(1024*t+j) + e
    x = np.asarray(o_core).reshape(2, 64, NTILES, 2, W)  # (h, b, t, e, j)
    x = x.transpose(1, 0, 2, 4, 3)  # (b, h, t, j, e)
    return np.ascontiguousarray(x.reshape(64, NT)).astype(np.float32)


def run(inputs, trace=False):
    nc = make_nc()
    p = np.asarray(inputs["p"], dtype=np.float32)
    hr = np.asarray(inputs["h_raw"], dtype=np.float32)
    in_maps = []
    for i in range(NCORES):
        sl = slice(i * BLOC, (i + 1) * BLOC)
        in_maps.append({"p": _stage_p(p[sl]), "h_raw": _stage_hr(hr[sl])})
    res = run_bass_kernel_spmd(nc, in_maps, core_ids=list(range(NCORES)), trace=trace)
    out = np.concatenate(
        [_unstage_out(res.results[i]["out"]) for i in range(NCORES)], axis=0
    )
    return out, res


def kernel(h, t, p, h_raw):
    out, _ = run({"p": p, "h_raw": h_raw})
    return out
